# revision 20
# baseline (speedup 1.0000x reference)
"""Multi-head causal attention (B=2, S=2048, D=1024, H=16) on 8 trn2 cores.

Sharding: core c handles batch b = c // 4 and head group g = c % 4 (4 heads,
256 feature columns). Each core computes its heads' attention context and a
partial output projection (ctx_g @ Wo[rows_g]); the host sums the 4 partials
per batch and adds bo.

v3 (all matmul operands bf16, fp32 psum accumulation):
- Demand-ordered DMAs (wq k0 first, wo late): first matmul starts ~2us in.
- Attention is sq-tile-major; projection slices and output-projection chunks
  are interleaved into the attention pair stream at matmul-chain granularity,
  so the PE always has independent dense work to hide the exp/ACT latency and
  the HAM activity monitor never downclocks.
- Within a sq-tile, ski pairs share one 2-bank psum + one wide exp ACTIVATE;
  the PV matmuls of pair i are issued after scores+exp of pair i+1.
- Causal diag masking via DVE multiply with a precomputed triangular bf16
  mask. Evacuation copies spread across ACT (qt) and DVE (kt, v, out).
- Softmax denominator: ones column folded into V stationary; reciprocal runs
  on a DMA-scattered [128,4] layout (all DVE lanes), broadcast back across
  partitions via a DRAM round trip. Output partials are written bf16 and
  summed on the host in fp32.
"""

import os
import sys
import types
from contextlib import ExitStack

import numpy as np
import ml_dtypes

import concourse.bacc as bacc
import concourse.bass as bass
import concourse.mybir as mybir
import concourse.tile as tile
from concourse.bass_utils import run_bass_kernel_spmd


def _install_ntff_hook():
    """The agent image's antenv lacks axon_hooks, so trn_boot's NTFF hook
    install degrades silently. Recreate the module + hook so trace=True works."""
    if "antenv.axon_hooks" in sys.modules:
        return
    try:
        mod = types.ModuleType("antenv.axon_hooks")
        holder = [None]
        mod.set_axon_ntff_profile_hook = lambda h: holder.__setitem__(0, h)
        mod.get_axon_ntff_profile_hook = lambda: holder[0]
        from trn_agent_boot.trn_boot import _ntff_profile_via_ctypes

        hook = _ntff_profile_via_ctypes("/opt/axon/libaxon_pjrt.so")
        if hook is None:
            return
        mod.set_axon_ntff_profile_hook(hook)
        sys.modules["antenv.axon_hooks"] = mod
    except Exception:
        pass

B, S, D, H, HD = 2, 2048, 1024, 16, 64
NCORES = 8
GROUPS = 4          # head groups (cores) per batch
HC = H // GROUPS    # heads per core
DG = HC * HD        # feature columns per core (256)
P = 128
KSUB = D // P       # 8 contraction subtiles for the projections
SQT = 512           # sq tile width (free dim of scores/ctx matmuls)
NSQ = S // SQT      # 4
NST = S // P        # 16 s subtiles of 128
F32 = mybir.dt.float32
BF16 = mybir.dt.bfloat16
BFNP = ml_dtypes.bfloat16

_CACHE = {}


def _mha_tile_kernel(tc, xT, wq, wk, wv, wo, out):
    nc = tc.nc
    scale = 1.0 / np.sqrt(np.float32(HD))

    with ExitStack() as ctx:
        consts = ctx.enter_context(tc.tile_pool(name="consts", bufs=1))
        dramp = ctx.enter_context(tc.tile_pool(name="dramp", bufs=3, space="DRAM"))
        # PSUM (8 banks): attn scores 2x[128,1024] + proj/outproj 2x[128,512]
        # + ctx accumulators 2x[128,512]
        psp = ctx.enter_context(tc.tile_pool(name="psp", bufs=2, space="PSUM"))
        xnp = ctx.enter_context(tc.tile_pool(name="xnp", bufs=3))
        ptp = ctx.enter_context(tc.tile_pool(name="ptp", bufs=6))
        smalls = ctx.enter_context(tc.tile_pool(name="smalls", bufs=3))
        outp = ctx.enter_context(tc.tile_pool(name="outp", bufs=3))

        # --- persistent SBUF tensors ---
        wq_sb = consts.tile([P, KSUB, DG], BF16)
        wk_sb = consts.tile([P, KSUB, DG], BF16)
        wv_sb = consts.tile([P, KSUB, DG], BF16)
        wo_sb = consts.tile([P, DG // P, D], BF16)
        qt_sb = consts.tile([P, DG // P, S], BF16)   # Q^T: head h at [64*(h%2):, h//2, :]
        # K^T zero-padded per head: head h's 64 rows at [64*(h%2):, h, :], the
        # other 64 rows 0 so score matmuls contract over K=128 (keeps the PE's
        # HAM activity monitor at full clock).
        kt_sb = consts.tile([P, HC, S], BF16)
        # V with the ones column baked in, per s-subtile and head:
        #   even h: [V(64) | 1 | 0(63)]  -> ctx rows 0-63, denom row 64
        #   odd  h: [1 | 0(63) | V(64)]  -> denom row 0, ctx rows 64-127
        v_sb = consts.tile([P, NST, HC, P], BF16)
        ctxt_sb = consts.tile([P, DG // P, S], BF16)  # normalized ctx^T, qt layout

        # demand-ordered input DMAs: wq k0 chunk, then x slice 0, then the rest
        nc.sync.dma_start(out=wq_sb[:, 0:1, :], in_=wq[:, 0:1, :])
        nc.sync.dma_start(out=wq_sb[:, 1:KSUB, :], in_=wq[:, 1:KSUB, :])

        def load_x(n):
            xn = xnp.tile([P, KSUB, SQT], BF16, tag="xT", name=f"xn_{n}")
            for k in range(KSUB):
                nc.sync.dma_start(
                    out=xn[:, k, :], in_=xT[k * P : (k + 1) * P, n * SQT : (n + 1) * SQT]
                )
            return xn

        xn0 = load_x(0)
        nc.sync.dma_start(out=wk_sb[:, 0:1, :], in_=wk[:, 0:1, :])
        nc.sync.dma_start(out=wk_sb[:, 1:KSUB, :], in_=wk[:, 1:KSUB, :])
        nc.sync.dma_start(out=wv_sb[:, 0:1, :], in_=wv[:, 0:1, :])
        nc.sync.dma_start(out=wv_sb[:, 1:KSUB, :], in_=wv[:, 1:KSUB, :])

        # --- small constants + one-time pad init (off the per-slice path) ---
        zrow = consts.tile([P, SQT], F32, tag="zrow", bufs=1)
        nc.vector.memset(zrow, 0.0)
        osc = consts.tile([P, 1], F32, tag="osc", bufs=1)
        nc.vector.memset(osc, 1.0)
        mskf = consts.tile([P, P], F32, tag="mskf", bufs=1)
        nc.vector.memset(mskf, 1.0)
        # keep entries with col >= row (sk <= sq), zero the rest
        nc.gpsimd.affine_select(
            out=mskf, in_=mskf, pattern=[[1, P]],
            compare_op=mybir.AluOpType.is_ge, fill=0.0,
            base=0, channel_multiplier=-1,
        )
        msk = consts.tile([P, P], BF16, tag="msk", bufs=1)
        nc.vector.tensor_copy(out=msk, in_=mskf)
        # kt pads: the 64 unused partitions per head stay zero forever
        nc.vector.tensor_copy(
            out=kt_sb[64:P, 0::2, :].rearrange("p h (a b) -> p h a b", b=SQT),
            in_=zrow[64:P, None, None, :].to_broadcast((64, 2, S // SQT, SQT)),
        )
        nc.vector.tensor_copy(
            out=kt_sb[0:64, 1::2, :].rearrange("p h (a b) -> p h a b", b=SQT),
            in_=zrow[0:64, None, None, :].to_broadcast((64, 2, S // SQT, SQT)),
        )
        # v pads + ones columns
        nc.vector.tensor_copy(
            out=v_sb[:, :, 0:HC:2, HD + 1 : P],
            in_=zrow[:, None, None, 0 : P - HD - 1].to_broadcast((P, NST, 2, P - HD - 1)),
        )
        nc.vector.tensor_copy(
            out=v_sb[:, :, 1:HC:2, 1:HD],
            in_=zrow[:, None, None, 0 : HD - 1].to_broadcast((P, NST, 2, HD - 1)),
        )
        for h in range(HC):
            oc = HD if h % 2 == 0 else 0
            nc.vector.tensor_copy(
                out=v_sb[:, :, h, oc : oc + 1],
                in_=osc[:, None, :].to_broadcast((P, NST, 1)),
            )

        def proj_chunks(n, xn):
            """Projection slice n as 6 independent PE chunks (Q m0, Q m1,
            K m0, K m1, V half0, V half1), each an 8..16-matmul psum chain."""
            nsl = slice(n * SQT, (n + 1) * SQT)
            for m in range(DG // P):
                ps = psp.tile([P, SQT], F32, tag="mm1b", name=f"qp_{n}_{m}")
                for k in range(KSUB):
                    nc.tensor.matmul(
                        ps, lhsT=wq_sb[:, k, m * P : (m + 1) * P], rhs=xn[:, k, :],
                        start=(k == 0), stop=(k == KSUB - 1),
                    )
                nc.scalar.copy(out=qt_sb[:, m, nsl], in_=ps)
                yield
            for m in range(DG // P):
                ps = psp.tile([P, SQT], F32, tag="mm1b", name=f"kp_{n}_{m}")
                for k in range(KSUB):
                    nc.tensor.matmul(
                        ps, lhsT=wk_sb[:, k, m * P : (m + 1) * P], rhs=xn[:, k, :],
                        start=(k == 0), stop=(k == KSUB - 1),
                    )
                nc.vector.tensor_copy(out=kt_sb[0:64, 2 * m, nsl], in_=ps[0:64, :])
                nc.vector.tensor_copy(out=kt_sb[64:P, 2 * m + 1, nsl], in_=ps[64:P, :])
                yield
            for half in range(2):
                ps = psp.tile([P, SQT], F32, tag="mm1b", name=f"vp_{n}_{half}")
                for j in range(2):
                    sst = 2 * half + j
                    for k in range(KSUB):
                        nc.tensor.matmul(
                            ps[:, j * DG : (j + 1) * DG],
                            lhsT=xn[:, k, sst * P : (sst + 1) * P],
                            rhs=wv_sb[:, k, :],
                            start=(k == 0), stop=(k == KSUB - 1),
                        )
                sta = n * (SQT // P) + 2 * half
                psv = ps.rearrange("p (t h d) -> p t h d", h=HC, d=HD)
                nc.vector.tensor_copy(
                    out=v_sb[:, sta : sta + 2, 0:HC:2, 0:HD], in_=psv[:, :, 0:HC:2, :]
                )
                nc.vector.tensor_copy(
                    out=v_sb[:, sta : sta + 2, 1:HC:2, HD:P], in_=psv[:, :, 1:HC:2, :]
                )
                yield

        def emit_norm(sqt, cp, nh):
            sq0 = sqt * SQT
            nhm, nhp = nh // 2, 64 * (nh % 2)
            ncr = 0 if nh % 2 == 0 else 64
            ndr = 64 if nh % 2 == 0 else 0
            # scatter the psum denom row across partitions so reciprocal uses
            # all DVE lanes, then broadcast 1/den back via a DRAM round trip
            rt = smalls.tile([1, SQT], F32, tag="rt", name=f"rt_{sqt}_{nh}")
            nc.scalar.copy(out=rt, in_=cp[ndr : ndr + 1, :])
            spread = smalls.tile([P, SQT // P], F32, tag="spread", name=f"sp_{sqt}_{nh}")
            nc.sync.dma_start(out=spread, in_=rt)
            nc.vector.reciprocal(out=spread, in_=spread)
            rec_d = dramp.tile([1, SQT], F32, tag="rec", name=f"rec_{sqt}_{nh}")
            nc.sync.dma_start(
                out=rec_d.rearrange("a (p f) -> (a p) f", p=P), in_=spread
            )
            bcast = smalls.tile([P, SQT], F32, tag="bcast", name=f"bc_{sqt}_{nh}")
            rec_b = bass.AP(
                tensor=rec_d.tensor, offset=rec_d.offset,
                ap=[[0, 64]] + [list(p) for p in rec_d.ap[1:]],
            )
            nc.sync.dma_start(out=bcast[ncr : ncr + 64, :], in_=rec_b)
            nc.vector.tensor_tensor(
                ctxt_sb[nhp : nhp + 64, nhm, sq0 : sq0 + SQT],
                cp[ncr : ncr + 64, :],
                bcast[ncr : ncr + 64, :],
                mybir.AluOpType.mult,
            )

        def attention_pairs(sqt):
            """sq-tile sqt, all 4 heads; yields at pair boundaries. PV of pair
            i is emitted after scores+exp of pair i+1 (also across heads)."""
            sq0 = sqt * SQT
            nsk = 4 * sqt + 4
            pending = None   # (infos, pt, cpsum, head) awaiting PV
            norm_q = []      # cpsum awaiting norm emission

            def emit_pv(p):
                infos, pt, cpsum, h_own = p
                for ski, w0, base in infos:
                    nc.tensor.matmul(
                        cpsum[:, w0:],
                        lhsT=v_sb[:, ski, h_own, :],
                        rhs=pt[:, base + w0 : base + SQT],
                        start=(ski == 0), stop=(ski == nsk - 1),
                    )

            for h in range(HC):
                hm = h // 2
                cpsum = psp.tile([P, SQT], F32, tag="ctx", name=f"ctx_{sqt}_{h}")
                for sk0 in range(0, nsk, 2):
                    spsum = psp.tile([P, 2 * SQT], F32, tag="s", name=f"s_{sqt}_{h}_{sk0}")
                    pt = ptp.tile([P, 2 * SQT], BF16, tag="pt", name=f"pt_{sqt}_{h}_{sk0}")
                    infos = []
                    for jj in range(2):
                        ski = sk0 + jj
                        diag = ski >= 4 * sqt
                        w0 = (128 * ski - sq0) if diag else 0
                        base = jj * SQT
                        nc.tensor.matmul(
                            spsum[:, base + w0 : base + SQT],
                            lhsT=kt_sb[:, h, ski * P : (ski + 1) * P],
                            rhs=qt_sb[:, hm, sq0 + w0 : sq0 + SQT],
                            start=True, stop=True,
                        )
                        infos.append((ski, w0, base, diag))
                    w0g = infos[0][1]
                    nc.scalar.activation(
                        out=pt[:, w0g : 2 * SQT], in_=spsum[:, w0g : 2 * SQT],
                        func=mybir.ActivationFunctionType.Exp,
                        bias=0.0, scale=float(scale),
                    )
                    for ski, w0, base, diag in infos:
                        if diag:
                            nc.vector.tensor_tensor(
                                pt[:, base + w0 : base + w0 + P],
                                pt[:, base + w0 : base + w0 + P],
                                msk, mybir.AluOpType.mult,
                            )
                    if pending is not None:
                        emit_pv(pending)
                        if pending[0][-1][0] == nsk - 1:  # closed a head's chain
                            norm_q.append((pending[2], pending[3]))
                    while norm_q:
                        cp, nh = norm_q.pop(0)
                        emit_norm(sqt, cp, nh)
                    pending = ([(ski, w0, base) for ski, w0, base, _ in infos], pt, cpsum, h)
                    yield
            if pending is not None:
                emit_pv(pending)
                emit_norm(sqt, pending[2], pending[3])

        def outproj_chunks(sqt, sts=None):
            """Output projection chunks for s rows of sq-tile sqt."""
            for st in sts if sts is not None else range(4 * sqt, 4 * sqt + 4):
                ot = outp.tile([P, D], BF16, tag="out", name=f"ot_{st}")
                for nn in range(D // SQT):
                    ps = psp.tile([P, SQT], F32, tag="mm1b", name=f"op_{st}_{nn}")
                    for k in range(DG // P):
                        nc.tensor.matmul(
                            ps,
                            lhsT=ctxt_sb[:, k, st * P : (st + 1) * P],
                            rhs=wo_sb[:, k, nn * SQT : (nn + 1) * SQT],
                            start=(k == 0), stop=(k == DG // P - 1),
                        )
                    nc.vector.tensor_copy(
                        out=ot[:, nn * SQT : (nn + 1) * SQT], in_=ps
                    )
                    if nn == D // SQT - 1:
                        nc.scalar.dma_start(out=out[st * P : (st + 1) * P, :], in_=ot)
                    yield

        # --- main schedule ---
        import itertools

        def drive(att, fills):
            """att: (generator, n_yields); fills: list of (generator, n_yields).
            Spreads fill chunks evenly across attention blocks."""
            agen, na = att
            fgen = itertools.chain(*[g for g, _ in fills])
            nf = sum(n for _, n in fills)
            acc = 0.0
            step = nf / na if na else 0.0
            done_f = 0
            for _ in agen:
                acc += step
                while done_f < int(acc + 1e-9):
                    if next(fgen, None) is None:
                        break
                    done_f += 1
            for _ in fgen:
                pass

        # fill the pipe: slice 0 projections run dense
        for _ in proj_chunks(0, xn0):
            pass
        xn1 = load_x(1)
        xn2 = load_x(2)
        drive((attention_pairs(0), 8), [(proj_chunks(1, xn1), 6)])
        nc.sync.dma_start(out=wo_sb, in_=wo)
        xn3 = load_x(3)
        drive((attention_pairs(1), 16), [(proj_chunks(2, xn2), 6), (outproj_chunks(0), 8)])
        drive((attention_pairs(2), 24), [(proj_chunks(3, xn3), 6), (outproj_chunks(1), 8)])
        drive((attention_pairs(3), 32), [(outproj_chunks(2), 8)])
        for _ in outproj_chunks(3):
            pass


def build_nc():
    if "nc" in _CACHE:
        return _CACHE["nc"]
    nc = bacc.Bacc("TRN2", target_bir_lowering=False, debug=False, num_devices=NCORES)
    xT = nc.dram_tensor("xT", (D, S), BF16, kind="ExternalInput").ap()
    wq = nc.dram_tensor("wq", (P, KSUB, DG), BF16, kind="ExternalInput").ap()
    wk = nc.dram_tensor("wk", (P, KSUB, DG), BF16, kind="ExternalInput").ap()
    wv = nc.dram_tensor("wv", (P, KSUB, DG), BF16, kind="ExternalInput").ap()
    wo = nc.dram_tensor("wo", (P, DG // P, D), BF16, kind="ExternalInput").ap()
    out = nc.dram_tensor("out", (S, D), BF16, kind="ExternalOutput").ap()
    with tile.TileContext(nc) as tc:
        _mha_tile_kernel(tc, xT, wq, wk, wv, wo, out)
    nc.compile()
    _CACHE["nc"] = nc
    return nc


def make_in_maps(x, Wq, Wk, Wv, Wo):
    x = np.asarray(x, np.float32)
    xTb = [np.ascontiguousarray(x[b].T).astype(BFNP) for b in range(B)]
    wqs, wks, wvs, wos = [], [], [], []
    for g in range(GROUPS):
        cols = slice(g * DG, (g + 1) * DG)

        def wslice(W):
            # [D, DG] -> [128, KSUB, DG] with [p, k, m] = W[k*128+p, m]
            return np.ascontiguousarray(
                np.asarray(W, np.float32)[:, cols].reshape(KSUB, P, DG).transpose(1, 0, 2)
            ).astype(BFNP)

        wqs.append(wslice(Wq))
        wks.append(wslice(Wk))
        wvs.append(wslice(Wv))
        wos.append(
            np.ascontiguousarray(
                np.asarray(Wo, np.float32)[cols, :].reshape(DG // P, P, D).transpose(1, 0, 2)
            ).astype(BFNP)
        )
    in_maps = []
    for c in range(NCORES):
        b, g = c // GROUPS, c % GROUPS
        in_maps.append(
            {"xT": xTb[b], "wq": wqs[g], "wk": wks[g], "wv": wvs[g], "wo": wos[g]}
        )
    return in_maps


def kernel(x, Wq, Wk, Wv, Wo, bo):
    nc = build_nc()
    in_maps = make_in_maps(x, Wq, Wk, Wv, Wo)
    trace = bool(int(os.environ.get("MHA_TRACE", "0")))
    if trace:
        _install_ntff_hook()
    res = run_bass_kernel_spmd(
        nc, in_maps, core_ids=list(range(NCORES)), trace=trace,
        trace_cores=list(range(NCORES)) if trace else None,
    )
    _CACHE["last_results"] = res
    bo = np.asarray(bo, np.float32)
    out = np.zeros((B, S, D), np.float32)
    for c in range(NCORES):
        out[c // GROUPS] += res.results[c]["out"].astype(np.float32)
    out += bo[None, None, :]
    return out


# revision 21
# speedup vs baseline: 1.0724x; 1.0724x over previous
"""Multi-head causal attention (B=2, S=2048, D=1024, H=16) on 8 trn2 cores.

Sharding: core c handles batch b = c // 4 and head group g = c % 4 (4 heads,
256 feature columns). Each core computes its heads' attention context and a
partial output projection (ctx_g @ Wo[rows_g]); the host sums the 4 partials
per batch and adds bo.

v3 (all matmul operands bf16, fp32 psum accumulation):
- Demand-ordered DMAs (wq k0 first, wo late): first matmul starts ~2us in.
- Attention is sq-tile-major; projection slices and output-projection chunks
  are interleaved into the attention pair stream at matmul-chain granularity,
  so the PE always has independent dense work to hide the exp/ACT latency and
  the HAM activity monitor never downclocks.
- Within a sq-tile, ski pairs share one 2-bank psum + one wide exp ACTIVATE;
  the PV matmuls of pair i are issued after scores+exp of pair i+1.
- Causal diag masking via DVE multiply with a precomputed triangular bf16
  mask. Evacuation copies spread across ACT (qt) and DVE (kt, v, out).
- Softmax denominator: ones column folded into V stationary; reciprocal runs
  on a DMA-scattered [128,4] layout (all DVE lanes), broadcast back across
  partitions via a DRAM round trip. Output partials are written bf16 and
  summed on the host in fp32.
"""

import os
import sys
import types
from contextlib import ExitStack

import numpy as np
import ml_dtypes

import concourse.bacc as bacc
import concourse.bass as bass
import concourse.mybir as mybir
import concourse.tile as tile
from concourse.bass_utils import run_bass_kernel_spmd


def _install_ntff_hook():
    """The agent image's antenv lacks axon_hooks, so trn_boot's NTFF hook
    install degrades silently. Recreate the module + hook so trace=True works."""
    if "antenv.axon_hooks" in sys.modules:
        return
    try:
        mod = types.ModuleType("antenv.axon_hooks")
        holder = [None]
        mod.set_axon_ntff_profile_hook = lambda h: holder.__setitem__(0, h)
        mod.get_axon_ntff_profile_hook = lambda: holder[0]
        from trn_agent_boot.trn_boot import _ntff_profile_via_ctypes

        hook = _ntff_profile_via_ctypes("/opt/axon/libaxon_pjrt.so")
        if hook is None:
            return
        mod.set_axon_ntff_profile_hook(hook)
        sys.modules["antenv.axon_hooks"] = mod
    except Exception:
        pass

B, S, D, H, HD = 2, 2048, 1024, 16, 64
NCORES = 8
GROUPS = 4          # head groups (cores) per batch
HC = H // GROUPS    # heads per core
DG = HC * HD        # feature columns per core (256)
P = 128
KSUB = D // P       # 8 contraction subtiles for the projections
SQT = 512           # sq tile width (free dim of scores/ctx matmuls)
NSQ = S // SQT      # 4
NST = S // P        # 16 s subtiles of 128
F32 = mybir.dt.float32
BF16 = mybir.dt.bfloat16
BFNP = ml_dtypes.bfloat16

_CACHE = {}


def _mha_tile_kernel(tc, xT, wq, wk, wv, wo, out):
    nc = tc.nc
    scale = 1.0 / np.sqrt(np.float32(HD))

    with ExitStack() as ctx:
        consts = ctx.enter_context(tc.tile_pool(name="consts", bufs=1))
        dramp = ctx.enter_context(tc.tile_pool(name="dramp", bufs=3, space="DRAM"))
        # PSUM (8 banks): attn scores 2x[128,1024] + proj/outproj 2x[128,512]
        # + ctx accumulators 2x[128,512]
        psp = ctx.enter_context(tc.tile_pool(name="psp", bufs=2, space="PSUM"))
        xnp = ctx.enter_context(tc.tile_pool(name="xnp", bufs=3))
        ptp = ctx.enter_context(tc.tile_pool(name="ptp", bufs=6))
        smalls = ctx.enter_context(tc.tile_pool(name="smalls", bufs=3))
        outp = ctx.enter_context(tc.tile_pool(name="outp", bufs=3))

        # --- persistent SBUF tensors ---
        wq_sb = consts.tile([P, KSUB, DG], BF16)
        wk_sb = consts.tile([P, KSUB, DG], BF16)
        wv_sb = consts.tile([P, KSUB, DG], BF16)
        wo_sb = consts.tile([P, DG // P, D], BF16)
        qt_sb = consts.tile([P, DG // P, S], BF16)   # Q^T: head h at [64*(h%2):, h//2, :]
        # K^T zero-padded per head: head h's 64 rows at [64*(h%2):, h, :], the
        # other 64 rows 0 so score matmuls contract over K=128 (keeps the PE's
        # HAM activity monitor at full clock).
        kt_sb = consts.tile([P, HC, S], BF16)
        # V with the ones column baked in, per s-subtile and head:
        #   even h: [V(64) | 1 | 0(63)]  -> ctx rows 0-63, denom row 64
        #   odd  h: [1 | 0(63) | V(64)]  -> denom row 0, ctx rows 64-127
        v_sb = consts.tile([P, NST, HC, P], BF16)
        ctxt_sb = consts.tile([P, DG // P, S], BF16)  # normalized ctx^T, qt layout

        # demand-ordered input DMAs: wq/x-slice-0 k-chunks interleaved so the
        # first projection chain is paced by neither stream alone
        def load_x(n):
            xn = xnp.tile([P, KSUB, SQT], BF16, tag="xT", name=f"xn_{n}")
            for k in range(KSUB):
                nc.sync.dma_start(
                    out=xn[:, k, :], in_=xT[k * P : (k + 1) * P, n * SQT : (n + 1) * SQT]
                )
            return xn

        xn0 = xnp.tile([P, KSUB, SQT], BF16, tag="xT", name="xn_0")
        for k in range(KSUB):
            nc.sync.dma_start(out=wq_sb[:, k : k + 1, :], in_=wq[:, k : k + 1, :])
            nc.sync.dma_start(
                out=xn0[:, k, :], in_=xT[k * P : (k + 1) * P, 0:SQT]
            )
        nc.sync.dma_start(out=wk_sb[:, 0:1, :], in_=wk[:, 0:1, :])
        nc.sync.dma_start(out=wk_sb[:, 1:KSUB, :], in_=wk[:, 1:KSUB, :])
        nc.sync.dma_start(out=wv_sb[:, 0:1, :], in_=wv[:, 0:1, :])
        nc.sync.dma_start(out=wv_sb[:, 1:KSUB, :], in_=wv[:, 1:KSUB, :])

        # --- small constants + one-time pad init (off the per-slice path) ---
        zrow = consts.tile([P, SQT], F32, tag="zrow", bufs=1)
        nc.vector.memset(zrow, 0.0)
        osc = consts.tile([P, 1], F32, tag="osc", bufs=1)
        nc.vector.memset(osc, 1.0)
        mskf = consts.tile([P, P], F32, tag="mskf", bufs=1)
        nc.vector.memset(mskf, 1.0)
        # keep entries with col >= row (sk <= sq), zero the rest
        nc.gpsimd.affine_select(
            out=mskf, in_=mskf, pattern=[[1, P]],
            compare_op=mybir.AluOpType.is_ge, fill=0.0,
            base=0, channel_multiplier=-1,
        )
        msk = consts.tile([P, P], BF16, tag="msk", bufs=1)
        nc.vector.tensor_copy(out=msk, in_=mskf)
        # kt pads: the 64 unused partitions per head stay zero forever
        nc.vector.tensor_copy(
            out=kt_sb[64:P, 0::2, :].rearrange("p h (a b) -> p h a b", b=SQT),
            in_=zrow[64:P, None, None, :].to_broadcast((64, 2, S // SQT, SQT)),
        )
        nc.vector.tensor_copy(
            out=kt_sb[0:64, 1::2, :].rearrange("p h (a b) -> p h a b", b=SQT),
            in_=zrow[0:64, None, None, :].to_broadcast((64, 2, S // SQT, SQT)),
        )
        # v pads + ones columns
        nc.vector.tensor_copy(
            out=v_sb[:, :, 0:HC:2, HD + 1 : P],
            in_=zrow[:, None, None, 0 : P - HD - 1].to_broadcast((P, NST, 2, P - HD - 1)),
        )
        nc.vector.tensor_copy(
            out=v_sb[:, :, 1:HC:2, 1:HD],
            in_=zrow[:, None, None, 0 : HD - 1].to_broadcast((P, NST, 2, HD - 1)),
        )
        for h in range(HC):
            oc = HD if h % 2 == 0 else 0
            nc.vector.tensor_copy(
                out=v_sb[:, :, h, oc : oc + 1],
                in_=osc[:, None, :].to_broadcast((P, NST, 1)),
            )

        def proj_chunks(n, xn):
            """Projection slice n as 6 independent PE chunks (Q m0, Q m1,
            K m0, K m1, V half0, V half1), each an 8..16-matmul psum chain."""
            nsl = slice(n * SQT, (n + 1) * SQT)
            for m in range(DG // P):
                ps = psp.tile([P, SQT], F32, tag="mm1b", name=f"qp_{n}_{m}")
                for k in range(KSUB):
                    nc.tensor.matmul(
                        ps, lhsT=wq_sb[:, k, m * P : (m + 1) * P], rhs=xn[:, k, :],
                        start=(k == 0), stop=(k == KSUB - 1),
                    )
                nc.scalar.copy(out=qt_sb[:, m, nsl], in_=ps)
                yield
            for m in range(DG // P):
                ps = psp.tile([P, SQT], F32, tag="mm1b", name=f"kp_{n}_{m}")
                for k in range(KSUB):
                    nc.tensor.matmul(
                        ps, lhsT=wk_sb[:, k, m * P : (m + 1) * P], rhs=xn[:, k, :],
                        start=(k == 0), stop=(k == KSUB - 1),
                    )
                nc.vector.tensor_copy(out=kt_sb[0:64, 2 * m, nsl], in_=ps[0:64, :])
                nc.vector.tensor_copy(out=kt_sb[64:P, 2 * m + 1, nsl], in_=ps[64:P, :])
                yield
            for half in range(2):
                ps = psp.tile([P, SQT], F32, tag="mm1b", name=f"vp_{n}_{half}")
                for j in range(2):
                    sst = 2 * half + j
                    for k in range(KSUB):
                        nc.tensor.matmul(
                            ps[:, j * DG : (j + 1) * DG],
                            lhsT=xn[:, k, sst * P : (sst + 1) * P],
                            rhs=wv_sb[:, k, :],
                            start=(k == 0), stop=(k == KSUB - 1),
                        )
                sta = n * (SQT // P) + 2 * half
                psv = ps.rearrange("p (t h d) -> p t h d", h=HC, d=HD)
                nc.vector.tensor_copy(
                    out=v_sb[:, sta : sta + 2, 0:HC:2, 0:HD], in_=psv[:, :, 0:HC:2, :]
                )
                nc.vector.tensor_copy(
                    out=v_sb[:, sta : sta + 2, 1:HC:2, HD:P], in_=psv[:, :, 1:HC:2, :]
                )
                yield

        def emit_norm(sqt, cp, nh):
            sq0 = sqt * SQT
            nhm, nhp = nh // 2, 64 * (nh % 2)
            ncr = 0 if nh % 2 == 0 else 64
            ndr = 64 if nh % 2 == 0 else 0
            # scatter the psum denom row across partitions so reciprocal uses
            # all DVE lanes, then broadcast 1/den back via a DRAM round trip
            rt = smalls.tile([1, SQT], F32, tag="rt", name=f"rt_{sqt}_{nh}")
            nc.scalar.copy(out=rt, in_=cp[ndr : ndr + 1, :])
            spread = smalls.tile([P, SQT // P], F32, tag="spread", name=f"sp_{sqt}_{nh}")
            nc.sync.dma_start(out=spread, in_=rt)
            nc.vector.reciprocal(out=spread, in_=spread)
            rec_d = dramp.tile([1, SQT], F32, tag="rec", name=f"rec_{sqt}_{nh}")
            nc.sync.dma_start(
                out=rec_d.rearrange("a (p f) -> (a p) f", p=P), in_=spread
            )
            bcast = smalls.tile([P, SQT], F32, tag="bcast", name=f"bc_{sqt}_{nh}")
            rec_b = bass.AP(
                tensor=rec_d.tensor, offset=rec_d.offset,
                ap=[[0, 64]] + [list(p) for p in rec_d.ap[1:]],
            )
            nc.sync.dma_start(out=bcast[ncr : ncr + 64, :], in_=rec_b)
            nc.vector.tensor_tensor(
                ctxt_sb[nhp : nhp + 64, nhm, sq0 : sq0 + SQT],
                cp[ncr : ncr + 64, :],
                bcast[ncr : ncr + 64, :],
                mybir.AluOpType.mult,
            )

        def attention_pairs(sqt):
            """sq-tile sqt, all 4 heads; yields at pair boundaries. PV of pair
            i is emitted after scores+exp of pair i+1 (also across heads)."""
            sq0 = sqt * SQT
            nsk = 4 * sqt + 4
            pending = None   # (infos, pt, cpsum, head) awaiting PV
            norm_q = []      # cpsum awaiting norm emission

            def emit_pv(p):
                infos, pt, cpsum, h_own = p
                for ski, w0, base in infos:
                    nc.tensor.matmul(
                        cpsum[:, w0:],
                        lhsT=v_sb[:, ski, h_own, :],
                        rhs=pt[:, base + w0 : base + SQT],
                        start=(ski == 0), stop=(ski == nsk - 1),
                    )

            horder = [2, 3, 0, 1] if sqt == NSQ - 1 else list(range(HC))
            for h in horder:
                hm = h // 2
                cpsum = psp.tile([P, SQT], F32, tag="ctx", name=f"ctx_{sqt}_{h}")
                for sk0 in range(0, nsk, 2):
                    spsum = psp.tile([P, 2 * SQT], F32, tag="s", name=f"s_{sqt}_{h}_{sk0}")
                    pt = ptp.tile([P, 2 * SQT], BF16, tag="pt", name=f"pt_{sqt}_{h}_{sk0}")
                    infos = []
                    for jj in range(2):
                        ski = sk0 + jj
                        diag = ski >= 4 * sqt
                        w0 = (128 * ski - sq0) if diag else 0
                        base = jj * SQT
                        nc.tensor.matmul(
                            spsum[:, base + w0 : base + SQT],
                            lhsT=kt_sb[:, h, ski * P : (ski + 1) * P],
                            rhs=qt_sb[:, hm, sq0 + w0 : sq0 + SQT],
                            start=True, stop=True,
                        )
                        infos.append((ski, w0, base, diag))
                    w0g = infos[0][1]
                    nc.scalar.activation(
                        out=pt[:, w0g : 2 * SQT], in_=spsum[:, w0g : 2 * SQT],
                        func=mybir.ActivationFunctionType.Exp,
                        bias=0.0, scale=float(scale),
                    )
                    for ski, w0, base, diag in infos:
                        if diag:
                            nc.vector.tensor_tensor(
                                pt[:, base + w0 : base + w0 + P],
                                pt[:, base + w0 : base + w0 + P],
                                msk, mybir.AluOpType.mult,
                            )
                    if pending is not None:
                        emit_pv(pending)
                        if pending[0][-1][0] == nsk - 1:  # closed a head's chain
                            norm_q.append((pending[2], pending[3]))
                    while norm_q:
                        cp, nh = norm_q.pop(0)
                        emit_norm(sqt, cp, nh)
                    pending = ([(ski, w0, base) for ski, w0, base, _ in infos], pt, cpsum, h)
                    yield
            if pending is not None:
                emit_pv(pending)
                emit_norm(sqt, pending[2], pending[3])

        def outproj_chunks(sqt, sts=None):
            """Output projection chunks for s rows of sq-tile sqt."""
            for st in sts if sts is not None else range(4 * sqt, 4 * sqt + 4):
                ot = outp.tile([P, D], BF16, tag="out", name=f"ot_{st}")
                for nn in range(D // SQT):
                    ps = psp.tile([P, SQT], F32, tag="mm1b", name=f"op_{st}_{nn}")
                    for k in range(DG // P):
                        nc.tensor.matmul(
                            ps,
                            lhsT=ctxt_sb[:, k, st * P : (st + 1) * P],
                            rhs=wo_sb[:, k, nn * SQT : (nn + 1) * SQT],
                            start=(k == 0), stop=(k == DG // P - 1),
                        )
                    nc.vector.tensor_copy(
                        out=ot[:, nn * SQT : (nn + 1) * SQT], in_=ps
                    )
                    if nn == D // SQT - 1:
                        nc.scalar.dma_start(out=out[st * P : (st + 1) * P, :], in_=ot)
                    yield

        # --- main schedule ---
        import itertools

        def drive(att, fills):
            """att: (generator, n_yields); fills: list of (generator, n_yields).
            Spreads fill chunks evenly across attention blocks."""
            agen, na = att
            fgen = itertools.chain(*[g for g, _ in fills])
            nf = sum(n for _, n in fills)
            acc = 0.0
            step = nf / na if na else 0.0
            done_f = 0
            for _ in agen:
                acc += step
                while done_f < int(acc + 1e-9):
                    if next(fgen, None) is None:
                        break
                    done_f += 1
            for _ in fgen:
                pass

        # fill the pipe: slice 0 projections run dense
        for _ in proj_chunks(0, xn0):
            pass
        xn1 = load_x(1)
        xn2 = load_x(2)
        drive((attention_pairs(0), 8), [(proj_chunks(1, xn1), 6)])
        nc.sync.dma_start(out=wo_sb, in_=wo)
        xn3 = load_x(3)
        drive((attention_pairs(1), 16), [(proj_chunks(2, xn2), 6), (outproj_chunks(0), 8)])
        drive((attention_pairs(2), 24), [(proj_chunks(3, xn3), 6), (outproj_chunks(1), 8)])
        drive((attention_pairs(3), 32), [(outproj_chunks(2), 8)])
        # tail: with sqt3 head order [2,3,0,1], ctxt slot 1 is ready before
        # slot 0's last norm; run all slot-1 matmuls during that norm chain
        for st0 in (12, 14):
            pss = []
            for st in (st0, st0 + 1):
                ps = psp.tile([P, 2 * SQT], F32, tag="s", name=f"opt_{st}")
                for nn in range(D // SQT):
                    nc.tensor.matmul(
                        ps[:, nn * SQT : (nn + 1) * SQT],
                        lhsT=ctxt_sb[:, 1, st * P : (st + 1) * P],
                        rhs=wo_sb[:, 1, nn * SQT : (nn + 1) * SQT],
                        start=True, stop=False,
                    )
                pss.append(ps)
            for st, ps in zip((st0, st0 + 1), pss):
                for nn in range(D // SQT):
                    nc.tensor.matmul(
                        ps[:, nn * SQT : (nn + 1) * SQT],
                        lhsT=ctxt_sb[:, 0, st * P : (st + 1) * P],
                        rhs=wo_sb[:, 0, nn * SQT : (nn + 1) * SQT],
                        start=False, stop=True,
                    )
            for st, ps in zip((st0, st0 + 1), pss):
                ot = outp.tile([P, D], BF16, tag="out", name=f"ot_{st}")
                nc.vector.tensor_copy(out=ot, in_=ps)
                nc.scalar.dma_start(out=out[st * P : (st + 1) * P, :], in_=ot)


def build_nc():
    if "nc" in _CACHE:
        return _CACHE["nc"]
    nc = bacc.Bacc("TRN2", target_bir_lowering=False, debug=False, num_devices=NCORES)
    xT = nc.dram_tensor("xT", (D, S), BF16, kind="ExternalInput").ap()
    wq = nc.dram_tensor("wq", (P, KSUB, DG), BF16, kind="ExternalInput").ap()
    wk = nc.dram_tensor("wk", (P, KSUB, DG), BF16, kind="ExternalInput").ap()
    wv = nc.dram_tensor("wv", (P, KSUB, DG), BF16, kind="ExternalInput").ap()
    wo = nc.dram_tensor("wo", (P, DG // P, D), BF16, kind="ExternalInput").ap()
    out = nc.dram_tensor("out", (S, D), BF16, kind="ExternalOutput").ap()
    with tile.TileContext(nc) as tc:
        _mha_tile_kernel(tc, xT, wq, wk, wv, wo, out)
    nc.compile()
    _CACHE["nc"] = nc
    return nc


def make_in_maps(x, Wq, Wk, Wv, Wo):
    x = np.asarray(x, np.float32)
    xTb = [np.ascontiguousarray(x[b].T).astype(BFNP) for b in range(B)]
    wqs, wks, wvs, wos = [], [], [], []
    for g in range(GROUPS):
        cols = slice(g * DG, (g + 1) * DG)

        def wslice(W):
            # [D, DG] -> [128, KSUB, DG] with [p, k, m] = W[k*128+p, m]
            return np.ascontiguousarray(
                np.asarray(W, np.float32)[:, cols].reshape(KSUB, P, DG).transpose(1, 0, 2)
            ).astype(BFNP)

        wqs.append(wslice(Wq))
        wks.append(wslice(Wk))
        wvs.append(wslice(Wv))
        wos.append(
            np.ascontiguousarray(
                np.asarray(Wo, np.float32)[cols, :].reshape(DG // P, P, D).transpose(1, 0, 2)
            ).astype(BFNP)
        )
    in_maps = []
    for c in range(NCORES):
        b, g = c // GROUPS, c % GROUPS
        in_maps.append(
            {"xT": xTb[b], "wq": wqs[g], "wk": wks[g], "wv": wvs[g], "wo": wos[g]}
        )
    return in_maps


def kernel(x, Wq, Wk, Wv, Wo, bo):
    nc = build_nc()
    in_maps = make_in_maps(x, Wq, Wk, Wv, Wo)
    trace = bool(int(os.environ.get("MHA_TRACE", "0")))
    if trace:
        _install_ntff_hook()
    res = run_bass_kernel_spmd(
        nc, in_maps, core_ids=list(range(NCORES)), trace=trace,
        trace_cores=list(range(NCORES)) if trace else None,
    )
    _CACHE["last_results"] = res
    bo = np.asarray(bo, np.float32)
    out = np.zeros((B, S, D), np.float32)
    for c in range(NCORES):
        out[c // GROUPS] += res.results[c]["out"].astype(np.float32)
    out += bo[None, None, :]
    return out
